# revision 16
# baseline (speedup 1.0000x reference)
"""TRN2 Bass kernel for nn_Attention_39316130628152.

Spatial self-attention: B=4, C=64, H=W=64 (N=4096 tokens), f32.
  q/k/v = 1x1conv(x);  out = v @ softmax(q^T k)^T

Sharding: 8 cores = (batch b in 0..3) x (query-half h in 0..1).
Each core handles 2048 queries x 4096 keys for one batch.

Key algebraic restructure vs the classic q/k projection:
  s_ij = q_i . k_j = x_i^T (Wq^T Wk) x_j + (bq^T Wk) x_j + (Wq^T bk) x_i + bq.bk
The last two terms are constant per query i, so they cancel in the
softmax normalization and are DROPPED.  With M^ = [Wq^T Wk ; bq^T Wk]
(65x64, host-precomputed) and x~_i = [x_i ; 1]:
  s_eff[i,j] = g_i . x_j   where g_i = M^T x~_i  (64-dim)
So the KEY side needs no projection at all (raw x), and only g (the
query side) is projected on device: 4 matmuls + 4 evacuations total
per core instead of 24 matmuls + 16 evacuations.

Per-core algorithm (ACT-exp-bound steady state ~64us):
  for each i-macro (512 queries) x j-pair (2x128 keys):
    sT[j,i] = xk_tile^T g     (fp16 row-tiled dup pair, concurrent in PE)
    p = exp(sT - 40)          (one ACT instr over 2 PSUM banks, bf16 out)
    U[e,i] += XT[e,:] p       (bf16, PSUM accum; XT packs x_hi|ones|x_lo
                               so U[64]=Z and hi/lo keeps near-fp32 accuracy)
  o[i, 0:64] = U^T WvT2 (unnormalized, includes Z*bv), o[i,64] = Z
  host divides by Z (normalization is scale-invariant, so the dropped
  per-i terms and the exp bias both cancel).

Output per core: [16, 128, 65] tiles of [i, c|Z]; host normalizes and
reassembles.
"""
import numpy as np
import ml_dtypes

import concourse.bacc as bacc
import concourse.mybir as mybir
import concourse.tile as tile
from concourse.bass_utils import run_bass_kernel_spmd

F32 = mybir.dt.float32
F32R = mybir.dt.float32r
F16 = mybir.dt.float16
BF16 = mybir.dt.bfloat16

B, C, HH, WW = 4, 64, 64, 64
N = HH * WW           # 4096 tokens
NQ = N // 2           # queries per core (2048)
IM = 512              # i-macro size
NIM = NQ // IM        # 4
JT = 128              # j-tile (keys per tile)
NPAIR = N // (2 * JT)  # 16 j-pairs per i-macro
NCH = IM // 128       # output chunks per i-macro (4)
EXP_BIAS = -40.0      # exp(s + EXP_BIAS); cancels in normalization

_NC_CACHE = {}


def build_nc():
    if "nc" in _NC_CACHE:
        return _NC_CACHE["nc"]
    nc = bacc.Bacc(None, target_bir_lowering=False)

    XK = nc.dram_tensor("XK", (128, N), F16, kind="ExternalInput")
    XQT = nc.dram_tensor("XQT", (C + 1, NQ), F16, kind="ExternalInput")
    MH = nc.dram_tensor("MH", (C + 1, C), F16, kind="ExternalInput")
    XT = nc.dram_tensor("XT", (128, N // JT, 128), BF16, kind="ExternalInput")
    # raw U rows per i-macro; host applies WvT2 + normalization (cheap).
    OUT = nc.dram_tensor("OUT", (NIM, 128, IM), F32, kind="ExternalOutput")

    with tile.TileContext(nc) as tc:
        with (
            tc.tile_pool(name="consts", bufs=1) as consts,
            tc.tile_pool(name="acts", bufs=1) as acts,
            tc.tile_pool(name="pexp", bufs=4) as pexp,
            tc.tile_pool(name="usbp", bufs=2) as usbp,
            tc.tile_pool(name="psS", bufs=3, space="PSUM") as psS,
            tc.tile_pool(name="psUA", bufs=1, space="PSUM") as psUA,
            tc.tile_pool(name="psUB", bufs=1, space="PSUM") as psUB,
        ):
            ebias_sb = consts.tile([128, 1], F32, tag="ebias")
            warm_sb = consts.tile([128, 512], BF16, tag="warm")
            nc.vector.memset(warm_sb, 0.0)
            nc.vector.memset(ebias_sb, EXP_BIAS)
            # dummy exp: pulls the ~2.7us ACT table load into the DMA head
            dume_sb = consts.tile([128, 2], F32, tag="dume")
            nc.scalar.activation(dume_sb[:, 0:1], ebias_sb[:, :],
                                 mybir.ActivationFunctionType.Exp)
            # warm-up matmuls: keep the PE busy through the DMA head so the
            # p-state ramp (0.65 -> 2.4 GHz after ~3us sustained) completes
            # before the first real matmuls.
            warm_ps = psS.tile([128, 1024], F32, tag="s", name="warm_ps")
            for _ in range(5):
                nc.tensor.matmul(warm_ps[:, 0:512], warm_sb[:, 0:128],
                                 warm_sb[:, :], start=True, stop=True)

            mh_sb = consts.tile([C + 1, C], F16, tag="mh")
            xq_sb = [consts.tile([C + 1, 512], F16, tag=f"xq{t}", name=f"xq{t}")
                     for t in range(4)]
            xk_sb = [consts.tile([128, 512], F16, tag=f"xk{t}", name=f"xk{t}")
                     for t in range(8)]
            xt_sb = [consts.tile([128, 8, 128], BF16, tag=f"xt{t}", name=f"xt{t}")
                     for t in range(4)]
            # Ring discipline: the Scalar queue must stay clear for ACTIVATE
            # (a queued DMA descriptor blocks exp for ~1us), so it carries
            # ONLY xq0.  sync (HWDGE) takes mh + xk + remaining xq in
            # first-use order; gpsimd (SWDGE, slow) takes the U-side XT.
            nc.scalar.dma_start(out=xq_sb[0], in_=XQT[:, 0:512])
            nc.sync.dma_start(out=mh_sb, in_=MH[:, :])
            nc.sync.dma_start(out=xk_sb[0], in_=XK[:, 0:512])
            nc.gpsimd.dma_start(out=xt_sb[0], in_=XT[:, 0:8, :])
            nc.sync.dma_start(out=xq_sb[1], in_=XQT[:, 512:1024])
            nc.sync.dma_start(out=xk_sb[1], in_=XK[:, 512:1024])
            nc.sync.dma_start(out=xk_sb[2], in_=XK[:, 1024:1536])
            nc.sync.dma_start(out=xq_sb[2], in_=XQT[:, 1024:1536])
            nc.sync.dma_start(out=xk_sb[3], in_=XK[:, 1536:2048])
            nc.sync.dma_start(out=xq_sb[3], in_=XQT[:, 1536:2048])
            for t in range(4, 8):
                nc.sync.dma_start(out=xk_sb[t],
                                  in_=XK[:, t * 512:(t + 1) * 512])
            for t in range(1, 4):
                nc.gpsimd.dma_start(out=xt_sb[t],
                                    in_=XT[:, t * 8:(t + 1) * 8, :])

            # g projection: g = M^T x~ for one 512-query chunk, duplicated
            # onto partitions 64-127 (col-tiled pair) so the scores matmuls
            # can row-tile over it.  K=65, single row-quadrant.
            g_sb = [acts.tile([128, 512], F16, tag=f"g{t}", name=f"g{t}")
                    for t in range(4)]

            def project_g(im):
                ps = psS.tile([128, 1024], F32, tag="s", name="gproj_ps")
                nc.tensor.matmul(ps[0:C, 0:512], mh_sb[:, :], xq_sb[im][:, :],
                                 start=True, stop=True, tile_position=(0, 0))
                nc.tensor.matmul(ps[C:128, 0:512], mh_sb[:, :], xq_sb[im][:, :],
                                 start=True, stop=True, tile_position=(0, 64))
                nc.vector.tensor_copy(g_sb[im][:, :], ps[:, 0:512])

            project_g(0)
            for im in range(NIM):
                uA_ps = psUA.tile([128, IM], F32, tag="ua")
                uB_ps = psUB.tile([128, IM], F32, tag="ub")
                gc = g_sb[im]
                for t in range(NPAIR):
                    jtA, jtB = 2 * t, 2 * t + 1
                    kc = xk_sb[t // 2]
                    ko = (t % 2) * 256
                    s_ps = psS.tile([128, 1024], F32, tag="s")
                    nc.tensor.matmul(
                        s_ps[:, 0:512],
                        kc[0:C, ko:ko + JT],
                        gc[0:C, :],
                        start=True, stop=True, tile_position=(0, 0))
                    nc.tensor.matmul(
                        s_ps[:, 512:1024],
                        kc[C:128, ko + JT:ko + 2 * JT],
                        gc[C:128, :],
                        start=True, stop=True, tile_position=(64, 0))
                    p_sb = pexp.tile([128, 1024], BF16, tag="p")
                    nc.scalar.activation(p_sb[:, :], s_ps[:, :],
                                         mybir.ActivationFunctionType.Exp,
                                         bias=ebias_sb[:, :])
                    # U split into token-halves on disjoint PE row quadrants
                    # and separate PSUM banks: the lo/hi pairs stream
                    # concurrently (~2x U throughput); host sums uA+uB.
                    st, sp = (t == 0), (t == NPAIR - 1)
                    for jt, po in ((jtA, 0), (jtB, 512)):
                        xtt = xt_sb[jt // 8]
                        nc.tensor.matmul(
                            uA_ps[:, :], xtt[0:64, jt % 8, :],
                            p_sb[0:64, po:po + 512],
                            start=(st and po == 0), stop=(sp and po == 512),
                            tile_position=(0, 0))
                        nc.tensor.matmul(
                            uB_ps[:, :], xtt[64:128, jt % 8, :],
                            p_sb[64:128, po:po + 512],
                            start=(st and po == 0), stop=(sp and po == 512),
                            tile_position=(64, 0))
                    if t == 2 and im < NIM - 1:
                        project_g(im + 1)
                ua_sb = usbp.tile([128, IM], F32, tag="ua_sb")
                nc.vector.tensor_copy(ua_sb[:, :], uA_ps[:, :])
                u_sb = usbp.tile([128, IM], F32, tag="u_sb")
                nc.vector.scalar_tensor_tensor(
                    u_sb[:, :], ua_sb[:, :], 1.0, uB_ps[:, :],
                    op0=mybir.AluOpType.mult, op1=mybir.AluOpType.add)
                nc.sync.dma_start(out=OUT[im, :, :], in_=u_sb)
    nc.finalize()
    _NC_CACHE["nc"] = nc
    return nc


def prep_inputs(x, Wq, bq, Wk, bk, Wv, bv):
    """Build the 8 per-core input maps (host-side numpy, cheap)."""
    f32 = np.float32
    f64 = np.float64
    # M^ = [Wq^T Wk ; bq^T Wk]: s_eff[i,j] = [x_i;1]^T M^ x_j
    mh = np.empty((C + 1, C), dtype=f64)
    mh[:C] = Wq.astype(f64).T @ Wk.astype(f64)
    mh[C] = bq.astype(f64) @ Wk.astype(f64)
    mh16 = mh.astype(np.float16)
    in_maps = []
    for core in range(8):
        b, h = core // 2, core % 2
        xb = np.ascontiguousarray(x[b].reshape(C, N)).astype(f32)
        x16 = xb.astype(np.float16)
        # key side: raw x duplicated on partitions for row-tiled scores
        xk = np.ascontiguousarray(np.concatenate([x16, x16], axis=0))
        # query side: x~ = [x ; 1] for this core's half
        xqt = np.concatenate(
            [x16[:, h * NQ:(h + 1) * NQ],
             np.ones((1, NQ), dtype=np.float16)], axis=0)
        xqt = np.ascontiguousarray(xqt)
        # XT[p, jt, :] = [x_hi(64) | 1 | x_lo(channels 0-62)] at token
        # jt*128+p; hi/lo bf16 split keeps the U matmul near-fp32 exact.
        x_hi = xb.astype(ml_dtypes.bfloat16)
        x_lo = (xb - x_hi.astype(f32)).astype(ml_dtypes.bfloat16)
        xt_full = np.zeros((C + 1 + 63, N), dtype=ml_dtypes.bfloat16)
        xt_full[:C] = x_hi
        xt_full[C] = 1.0
        xt_full[C + 1:] = x_lo[:C - 1]
        xt = np.ascontiguousarray(
            xt_full.T.reshape(N // JT, 128, 128).transpose(1, 0, 2))
        in_maps.append(dict(XK=xk, XQT=xqt, MH=mh16, XT=xt))
    return in_maps


def assemble_output(results, Wv, bv):
    """Host epilogue: U rows are [U_hi(64) | Z | U_lo(63)]; the output is
    (Wv @ (U_hi + U_lo-extended)) / Z + bv."""
    wv = Wv.astype(np.float64)
    out = np.empty((B, C, N), dtype=np.float32)
    for core in range(8):
        b, h = core // 2, core % 2
        u = results[core]["OUT"].astype(np.float64)  # [NIM, 128 e-rows, IM]
        u = u.transpose(1, 0, 2).reshape(128, NQ)
        num = wv @ u[:C] + wv[:, :C - 1] @ u[C + 1:]  # [C, NQ]
        z = u[C]
        out[b, :, h * NQ:(h + 1) * NQ] = ((num / z) + bv[:, None]).astype(
            np.float32)
    return out.reshape(B, C, HH, WW)


def kernel(x, Wq, bq, Wk, bk, Wv, bv, **run_kwargs):
    x = np.asarray(x, dtype=np.float32)
    nc = build_nc()
    in_maps = prep_inputs(np.asarray(x), np.asarray(Wq), np.asarray(bq),
                          np.asarray(Wk), np.asarray(bk),
                          np.asarray(Wv), np.asarray(bv))
    res = run_bass_kernel_spmd(nc, in_maps, core_ids=list(range(8)),
                               **run_kwargs)
    out = assemble_output(res.results, np.asarray(Wv), np.asarray(bv))
    if run_kwargs:
        return out, res
    return out


if __name__ == "__main__":
    rng = np.random.default_rng(0)
    s = 1.0 / np.sqrt(C)
    x = rng.standard_normal((B, C, HH, WW), dtype=np.float32)
    args = dict(
        x=x,
        Wq=(rng.standard_normal((C, C), dtype=np.float32) * s),
        bq=(rng.standard_normal(C, dtype=np.float32) * 0.01),
        Wk=(rng.standard_normal((C, C), dtype=np.float32) * s),
        bk=(rng.standard_normal(C, dtype=np.float32) * 0.01),
        Wv=(rng.standard_normal((C, C), dtype=np.float32) * s),
        bv=(rng.standard_normal(C, dtype=np.float32) * 0.01),
    )
    out = kernel(**args)
    print("kernel output:", out.shape, out.dtype)


# revision 19
# speedup vs baseline: 1.2856x; 1.2856x over previous
"""TRN2 Bass kernel for nn_Attention_39316130628152.

Spatial self-attention: B=4, C=64, H=W=64 (N=4096 tokens), f32.
  q/k/v = 1x1conv(x);  out = v @ softmax(q^T k)^T

Sharding: 8 cores = (batch b in 0..3) x (query-half h in 0..1).
Each core handles 2048 queries x 4096 keys for one batch.

Key algebraic restructure vs the classic q/k projection:
  s_ij = q_i . k_j = x_i^T (Wq^T Wk) x_j + (bq^T Wk) x_j + (Wq^T bk) x_i + bq.bk
The last two terms are constant per query i, so they cancel in the
softmax normalization and are DROPPED.  With M^ = [Wq^T Wk ; bq^T Wk]
(65x64, host-precomputed) and x~_i = [x_i ; 1]:
  s_eff[i,j] = g_i . x_j   where g_i = M^T x~_i  (64-dim)
So the KEY side needs no projection at all (raw x), and only g (the
query side) is projected on device: 4 matmuls + 4 evacuations total
per core instead of 24 matmuls + 16 evacuations.

Per-core algorithm (ACT-exp-bound steady state ~64us):
  for each i-macro (512 queries) x j-pair (2x128 keys):
    sT[j,i] = xk_tile^T g     (fp16 row-tiled dup pair, concurrent in PE)
    p = exp(sT - 40)          (one ACT instr over 2 PSUM banks, bf16 out)
    U[e,i] += XT[e,:] p       (bf16, PSUM accum; XT packs x_hi|ones|x_lo
                               so U[64]=Z and hi/lo keeps near-fp32 accuracy)
  o[i, 0:64] = U^T WvT2 (unnormalized, includes Z*bv), o[i,64] = Z
  host divides by Z (normalization is scale-invariant, so the dropped
  per-i terms and the exp bias both cancel).

Output per core: [16, 128, 65] tiles of [i, c|Z]; host normalizes and
reassembles.
"""
import numpy as np
import ml_dtypes

import concourse.bacc as bacc
import concourse.mybir as mybir
import concourse.tile as tile
from concourse.bass_utils import run_bass_kernel_spmd

F32 = mybir.dt.float32
F32R = mybir.dt.float32r
F16 = mybir.dt.float16
BF16 = mybir.dt.bfloat16

B, C, HH, WW = 4, 64, 64, 64
N = HH * WW           # 4096 tokens
NQ = N // 2           # queries per core (2048)
IM = 512              # i-macro size
NIM = NQ // IM        # 4
JT = 128              # j-tile (keys per tile)
NPAIR = N // (2 * JT)  # 16 j-pairs per i-macro
NCH = IM // 128       # output chunks per i-macro (4)
EXP_BIAS = -40.0      # exp(s + EXP_BIAS); cancels in normalization

_NC_CACHE = {}


def build_nc():
    if "nc" in _NC_CACHE:
        return _NC_CACHE["nc"]
    nc = bacc.Bacc(None, target_bir_lowering=False)

    XK = nc.dram_tensor("XK", (128, N), F16, kind="ExternalInput")
    XQT = nc.dram_tensor("XQT", (C + 1, NQ), F16, kind="ExternalInput")
    MH = nc.dram_tensor("MH", (C + 1, C), F16, kind="ExternalInput")
    XT = nc.dram_tensor("XT", (128, N // JT, 128), BF16, kind="ExternalInput")
    # raw U rows per i-macro; host applies WvT2 + normalization (cheap).
    OUT = nc.dram_tensor("OUT", (NIM, 128, IM), F32, kind="ExternalOutput")

    with tile.TileContext(nc) as tc:
        with (
            tc.tile_pool(name="consts", bufs=1) as consts,
            tc.tile_pool(name="acts", bufs=1) as acts,
            tc.tile_pool(name="pexp", bufs=4) as pexp,
            tc.tile_pool(name="usbp", bufs=2) as usbp,
            tc.tile_pool(name="psS", bufs=3, space="PSUM") as psS,
            tc.tile_pool(name="psU", bufs=2, space="PSUM") as psU,
        ):
            ebias_sb = consts.tile([128, 1], F32, tag="ebias")
            warm_sb = consts.tile([128, 512], BF16, tag="warm")
            nc.vector.memset(warm_sb, 0.0)
            nc.vector.memset(ebias_sb, EXP_BIAS)
            # dummy exp: pulls the ~2.7us ACT table load into the DMA head
            dume_sb = consts.tile([128, 2], F32, tag="dume")
            nc.scalar.activation(dume_sb[:, 0:1], ebias_sb[:, :],
                                 mybir.ActivationFunctionType.Exp)
            # warm-up matmuls: keep the PE busy through the DMA head so the
            # p-state ramp (0.65 -> 2.4 GHz after ~3us sustained) completes
            # before the first real matmuls.
            warm_ps = psS.tile([128, 1024], F32, tag="s", name="warm_ps")
            for _ in range(5):
                nc.tensor.matmul(warm_ps[:, 0:512], warm_sb[:, 0:128],
                                 warm_sb[:, :], start=True, stop=True)

            mh_sb = consts.tile([C + 1, C], F16, tag="mh")
            xq_sb = [consts.tile([C + 1, 512], F16, tag=f"xq{t}", name=f"xq{t}")
                     for t in range(4)]
            xk_sb = [consts.tile([128, 512], F16, tag=f"xk{t}", name=f"xk{t}")
                     for t in range(8)]
            xt_sb = [consts.tile([128, 8, 128], BF16, tag=f"xt{t}", name=f"xt{t}")
                     for t in range(4)]
            # Ring discipline: the Scalar queue must stay clear for ACTIVATE
            # (a queued DMA descriptor blocks exp for ~1us), so it carries
            # ONLY xq0.  sync (HWDGE) takes mh + xk + remaining xq in
            # first-use order; gpsimd (SWDGE, slow) takes the U-side XT.
            nc.scalar.dma_start(out=xq_sb[0], in_=XQT[:, 0:512])
            nc.sync.dma_start(out=mh_sb, in_=MH[:, :])
            nc.sync.dma_start(out=xk_sb[0], in_=XK[:, 0:512])
            nc.gpsimd.dma_start(out=xt_sb[0], in_=XT[:, 0:8, :])
            nc.sync.dma_start(out=xq_sb[1], in_=XQT[:, 512:1024])
            nc.sync.dma_start(out=xk_sb[1], in_=XK[:, 512:1024])
            nc.sync.dma_start(out=xk_sb[2], in_=XK[:, 1024:1536])
            nc.sync.dma_start(out=xq_sb[2], in_=XQT[:, 1024:1536])
            nc.sync.dma_start(out=xk_sb[3], in_=XK[:, 1536:2048])
            nc.sync.dma_start(out=xq_sb[3], in_=XQT[:, 1536:2048])
            for t in range(4, 8):
                nc.sync.dma_start(out=xk_sb[t],
                                  in_=XK[:, t * 512:(t + 1) * 512])
            for t in range(1, 4):
                nc.gpsimd.dma_start(out=xt_sb[t],
                                    in_=XT[:, t * 8:(t + 1) * 8, :])

            # g projection: g = M^T x~ for one 512-query chunk, duplicated
            # onto partitions 64-127 (col-tiled pair) so the scores matmuls
            # can row-tile over it.  K=65, single row-quadrant.
            g_sb = [acts.tile([128, 512], F16, tag=f"g{t}", name=f"g{t}")
                    for t in range(4)]

            def project_g(im):
                ps = psS.tile([128, 1024], F32, tag="s", name="gproj_ps")
                nc.tensor.matmul(ps[0:C, 0:512], mh_sb[:, :], xq_sb[im][:, :],
                                 start=True, stop=True, tile_position=(0, 0))
                nc.tensor.matmul(ps[C:128, 0:512], mh_sb[:, :], xq_sb[im][:, :],
                                 start=True, stop=True, tile_position=(0, 64))
                nc.vector.tensor_copy(g_sb[im][:, :], ps[:, 0:512])

            project_g(0)
            for im in range(NIM):
                u_ps = psU.tile([128, IM], F32, tag="u")
                gc = g_sb[im]
                for t in range(NPAIR):
                    jtA, jtB = 2 * t, 2 * t + 1
                    kc = xk_sb[t // 2]
                    ko = (t % 2) * 256
                    s_ps = psS.tile([128, 1024], F32, tag="s")
                    nc.tensor.matmul(
                        s_ps[:, 0:512],
                        kc[0:C, ko:ko + JT],
                        gc[0:C, :],
                        start=True, stop=True, tile_position=(0, 0))
                    nc.tensor.matmul(
                        s_ps[:, 512:1024],
                        kc[C:128, ko + JT:ko + 2 * JT],
                        gc[C:128, :],
                        start=True, stop=True, tile_position=(64, 0))
                    p_sb = pexp.tile([128, 1024], BF16, tag="p")
                    nc.scalar.activation(p_sb[:, :], s_ps[:, :],
                                         mybir.ActivationFunctionType.Exp,
                                         bias=ebias_sb[:, :])
                    nc.tensor.matmul(
                        u_ps[:, :], xt_sb[jtA // 8][:, jtA % 8, :],
                        p_sb[:, 0:512],
                        start=(t == 0), stop=False)
                    nc.tensor.matmul(
                        u_ps[:, :], xt_sb[jtB // 8][:, jtB % 8, :],
                        p_sb[:, 512:1024],
                        start=False, stop=(t == NPAIR - 1))
                    if t == 2 and im < NIM - 1:
                        project_g(im + 1)
                u_sb = usbp.tile([128, IM], F32, tag="u_sb")
                nc.vector.tensor_copy(u_sb[:, :], u_ps[:, :])
                nc.sync.dma_start(out=OUT[im, :, :], in_=u_sb)
    nc.finalize()
    _NC_CACHE["nc"] = nc
    return nc


def prep_inputs(x, Wq, bq, Wk, bk, Wv, bv):
    """Build the 8 per-core input maps (host-side numpy, cheap)."""
    f32 = np.float32
    f64 = np.float64
    # M^ = [Wq^T Wk ; bq^T Wk]: s_eff[i,j] = [x_i;1]^T M^ x_j
    mh = np.empty((C + 1, C), dtype=f64)
    mh[:C] = Wq.astype(f64).T @ Wk.astype(f64)
    mh[C] = bq.astype(f64) @ Wk.astype(f64)
    mh16 = mh.astype(np.float16)
    in_maps = []
    for core in range(8):
        b, h = core // 2, core % 2
        xb = np.ascontiguousarray(x[b].reshape(C, N)).astype(f32)
        x16 = xb.astype(np.float16)
        # key side: raw x duplicated on partitions for row-tiled scores
        xk = np.ascontiguousarray(np.concatenate([x16, x16], axis=0))
        # query side: x~ = [x ; 1] for this core's half
        xqt = np.concatenate(
            [x16[:, h * NQ:(h + 1) * NQ],
             np.ones((1, NQ), dtype=np.float16)], axis=0)
        xqt = np.ascontiguousarray(xqt)
        # XT[p, jt, :] = [x_hi(64) | 1 | x_lo(channels 0-62)] at token
        # jt*128+p; hi/lo bf16 split keeps the U matmul near-fp32 exact.
        x_hi = xb.astype(ml_dtypes.bfloat16)
        x_lo = (xb - x_hi.astype(f32)).astype(ml_dtypes.bfloat16)
        xt_full = np.zeros((C + 1 + 63, N), dtype=ml_dtypes.bfloat16)
        xt_full[:C] = x_hi
        xt_full[C] = 1.0
        xt_full[C + 1:] = x_lo[:C - 1]
        xt = np.ascontiguousarray(
            xt_full.T.reshape(N // JT, 128, 128).transpose(1, 0, 2))
        in_maps.append(dict(XK=xk, XQT=xqt, MH=mh16, XT=xt))
    return in_maps


def assemble_output(results, Wv, bv):
    """Host epilogue: U rows are [U_hi(64) | Z | U_lo(63)]; the output is
    (Wv @ (U_hi + U_lo-extended)) / Z + bv."""
    wv = Wv.astype(np.float64)
    out = np.empty((B, C, N), dtype=np.float32)
    for core in range(8):
        b, h = core // 2, core % 2
        u = results[core]["OUT"].astype(np.float64)  # [NIM, 128 e-rows, IM]
        u = u.transpose(1, 0, 2).reshape(128, NQ)
        num = wv @ u[:C] + wv[:, :C - 1] @ u[C + 1:]  # [C, NQ]
        z = u[C]
        out[b, :, h * NQ:(h + 1) * NQ] = ((num / z) + bv[:, None]).astype(
            np.float32)
    return out.reshape(B, C, HH, WW)


def kernel(x, Wq, bq, Wk, bk, Wv, bv, **run_kwargs):
    x = np.asarray(x, dtype=np.float32)
    nc = build_nc()
    in_maps = prep_inputs(np.asarray(x), np.asarray(Wq), np.asarray(bq),
                          np.asarray(Wk), np.asarray(bk),
                          np.asarray(Wv), np.asarray(bv))
    res = run_bass_kernel_spmd(nc, in_maps, core_ids=list(range(8)),
                               **run_kwargs)
    out = assemble_output(res.results, np.asarray(Wv), np.asarray(bv))
    if run_kwargs:
        return out, res
    return out


if __name__ == "__main__":
    rng = np.random.default_rng(0)
    s = 1.0 / np.sqrt(C)
    x = rng.standard_normal((B, C, HH, WW), dtype=np.float32)
    args = dict(
        x=x,
        Wq=(rng.standard_normal((C, C), dtype=np.float32) * s),
        bq=(rng.standard_normal(C, dtype=np.float32) * 0.01),
        Wk=(rng.standard_normal((C, C), dtype=np.float32) * s),
        bk=(rng.standard_normal(C, dtype=np.float32) * 0.01),
        Wv=(rng.standard_normal((C, C), dtype=np.float32) * s),
        bv=(rng.standard_normal(C, dtype=np.float32) * 0.01),
    )
    out = kernel(**args)
    print("kernel output:", out.shape, out.dtype)
